# revision 11
# baseline (speedup 1.0000x reference)
"""Trainium2 Bass kernel for MIGAttention (topk token masking + GQA attention).

Shapes (hardcoded): B=4, N=2048, C=1024, H=16 heads, HKV=4 kv-heads, DH=64,
keep-ratio 0.7 -> k = 1433 selected tokens per batch row.

Sharding: 8 cores = (batch b in 0..3) x (query-half h in 0..1).  Each core
receives x[b].T (bf16 hi/lo split) with token columns rolled by h*1024 so its
own query half occupies columns 0..1023 -> a single SPMD program for all
cores.  Each core computes the full gate+topk mask, compacts the 1433
selected tokens' K/V rows into a dense 1536-row DRAM buffer via an indexed
scatter (masked tokens go to a trash row), and runs GQA attention of its 1024
queries against the compact keys.  Masked keys contribute exp(0)=1 to the
softmax denominator analytically (+615 constant).

Precision: projections/attention in bf16 (fp32 psum accumulate).  The router
is computed in split-bf16 (x_hi@rw_hi + x_hi@rw_lo + x_lo@rw_hi, fp32
accumulate -> ~1e-5 logit error) so the 4-round threshold refinement
reproduces the reference top-k mask exactly (verified offline on the golden
inputs; window after 4 rounds = 8/128^4 = 3e-8).
"""

import contextlib
import sys

import numpy as np

if "/opt/trn_rl_repo" not in sys.path:
    sys.path.insert(0, "/opt/trn_rl_repo")

import concourse.bass as bass  # noqa: F401
import concourse.bass_isa as bass_isa
import concourse.mybir as mybir
from concourse import bacc
from concourse.tile import TileContext

F32 = mybir.dt.float32
F32R = mybir.dt.float32r
BF16 = mybir.dt.bfloat16
I32 = mybir.dt.int32
I16 = mybir.dt.int16
AF = mybir.ActivationFunctionType
ALU = mybir.AluOpType

B, N, C = 4, 2048, 1024
H, HKV, DH = 16, 4, 64
NQ = N // 2           # queries per core
KSEL = 1433           # max(1, int(N * 0.7))
CC = C // 128         # contraction chunks (8)
NCPT = 1536           # compact (padded) key count
KCC = NCPT // 128     # compact key chunks (12)
TRASH = NCPT          # scatter destination for masked tokens
KVW = 640             # kv row: K(4g x 64) | 4 x (64 v, 1 one, 31 zero)
VOFF, VSTR = 256, 96
N_ROUNDS = 4          # topk threshold refinement rounds
LO0, W0 = -4.0, 8.0   # initial logit search interval (logit std ~0.65)
ZCORR = float(N - KSEL)  # masked keys' exp(0) denominator contribution
# slot j holds q-heads (ORDER[2j], ORDER[2j+1]) on partition halves; the two
# heads come from kv groups (0,1) for j<4 and (2,3) for j>=4 so each matches
# one 64-partition half of a kt slot (row-tiled K=64 QK^T matmuls).
ORDER = [0, 4, 1, 5, 2, 6, 3, 7, 8, 12, 9, 13, 10, 14, 11, 15]


def _emit(nc, tc, ctx, io):
    xhi, xlo, rwh, rwl, wq, wkv, wo, sel8_d, l16_d, out_d = (
        io["xhi"], io["xlo"], io["rwh"], io["rwl"], io["wq"], io["wkv"],
        io["wo"], io["sel8"], io["l16e"], io["out"])

    # ---------------- long-lived pools ----------------
    const = ctx.enter_context(tc.tile_pool(name="const", bufs=1))
    small = ctx.enter_context(tc.tile_pool(name="small", bufs=1))
    big = ctx.enter_context(tc.tile_pool(name="big", bufs=1))
    dram = ctx.enter_context(tc.tile_pool(name="dram", bufs=1, space="DRAM"))

    # stack-ordered phase pools (closed LIFO): px > pw > pa > {plo|pkv|pq}
    px_ctx = contextlib.ExitStack()    # xhi (through projections)
    pw_ctx = contextlib.ExitStack()    # wq, wkv
    pa_ctx = contextlib.ExitStack()    # router/refinement scratch
    plo_ctx = contextlib.ExitStack()   # xlo (router only)
    pkv_ctx = contextlib.ExitStack()   # kv staging

    # ---------------- constants ----------------
    ones_row = const.tile([1, 128], F32)
    nc.vector.memset(ones_row, 1.0)
    iota128_i = const.tile([128, 1], I32)
    nc.gpsimd.iota(iota128_i, pattern=[[0, 1]], base=1, channel_multiplier=1)
    iota128 = const.tile([128, 1], F32)
    nc.vector.tensor_copy(iota128, iota128_i)
    sel8 = const.tile([16, CC, 128], F32R)
    nc.sync.dma_start(sel8, sel8_d.bitcast(F32R))
    l16e = const.tile([16, 16], F32)
    nc.sync.dma_start(l16e, l16_d)

    # ---------------- DRAM scratch ----------------
    kv_dram = dram.tile([NCPT + 1, KVW], BF16)
    m_dram = dram.tile([N], F32)
    g_dram = dram.tile([N], F32)
    pos_dram = dram.tile([N], I16)

    # ---------------- big loads ----------------
    px = px_ctx.enter_context(tc.tile_pool(name="px", bufs=1))
    pw = pw_ctx.enter_context(tc.tile_pool(name="pw", bufs=1))
    pa = pa_ctx.enter_context(tc.tile_pool(name="pa", bufs=1))
    pr_ctx = contextlib.ExitStack()    # router/bcast psum (pre-KV phase)
    psum_r = pr_ctx.enter_context(tc.tile_pool(name="psum_r", bufs=1,
                                               space="PSUM"))
    psum_b = pr_ctx.enter_context(tc.tile_pool(name="psum_b", bufs=2,
                                               space="PSUM"))
    plo = plo_ctx.enter_context(tc.tile_pool(name="plo", bufs=1))
    xhi_sb = px.tile([128, CC, N], BF16)
    xlo_sb = plo.tile([128, CC, N], BF16)
    for cc in range(CC):
        sl = slice(cc * 128, (cc + 1) * 128)
        nc.sync.dma_start(xhi_sb[:, cc, :], xhi[sl, :])
        nc.sync.dma_start(xlo_sb[:, cc, :], xlo[sl, :])
    rwh_sb = small.tile([128, CC], BF16)
    rwl_sb = small.tile([128, CC], BF16)
    for cc in range(CC):
        sl = slice(cc * 128, (cc + 1) * 128)
        nc.sync.dma_start(rwh_sb[:, cc:cc + 1], rwh[sl, :])
        nc.sync.dma_start(rwl_sb[:, cc:cc + 1], rwl[sl, :])
    wkv_sb = pw.tile([128, CC, 512], BF16)
    wq_sb = pw.tile([128, CC, C], BF16)
    for cc in range(CC):
        sl = slice(cc * 128, (cc + 1) * 128)
        nc.sync.dma_start(wkv_sb[:, cc, :], wkv[sl, :])
        nc.sync.dma_start(wq_sb[:, cc, :], wq[sl, :])

    # zero-fill compact kv buffer (pads + trash row stay zero; padded key
    # rows then have K=0 and a zero ones-column, so they are inert)
    zero_sb = small.tile([128, KVW], BF16)
    nc.vector.memset(zero_sb, 0.0)
    for i in range(12):
        nc.sync.dma_start(kv_dram[i * 128:(i + 1) * 128, :], zero_sb)
    nc.sync.dma_start(kv_dram[NCPT:NCPT + 1, :], zero_sb[0:1, :])

    # ---------------- router: logits in split-bf16 ----------------
    rps = psum_r.tile([1, N], F32)
    for cc in range(CC):
        for g in range(4):
            gs = slice(g * 512, (g + 1) * 512)
            nc.tensor.matmul(rps[:, gs], rwh_sb[:, cc:cc + 1],
                             xhi_sb[:, cc, gs], start=(cc == 0), stop=False)
            nc.tensor.matmul(rps[:, gs], rwl_sb[:, cc:cc + 1],
                             xhi_sb[:, cc, gs], start=False, stop=False)
            nc.tensor.matmul(rps[:, gs], rwh_sb[:, cc:cc + 1],
                             xlo_sb[:, cc, gs], start=False, stop=(cc == CC - 1))
    logits_sb = pa.tile([1, N], F32)
    nc.vector.tensor_copy(logits_sb, rps)
    plo_ctx.close()

    # replicate logits across partitions (fp32 K=1 broadcast matmuls)
    lrep = pa.tile([128, N], F32)
    for g in range(4):
        ps = psum_b.tile([128, 512], F32, tag="bcast")
        nc.tensor.matmul(ps, ones_row, logits_sb[:, g * 512:(g + 1) * 512],
                         start=True, stop=True)
        nc.vector.tensor_copy(lrep[:, g * 512:(g + 1) * 512], ps)

    # gate row (mask-independent; unblocks K/V eviction during refinement)
    grow = pa.tile([1, N], F32)
    nc.scalar.activation(grow, logits_sb, AF.Sigmoid)
    nc.sync.dma_start(g_dram, grow)
    g_v = small.tile([128, 16], F32)
    nc.sync.dma_start(g_v, g_dram.rearrange("(c p) -> p c", p=128))

    # ---------------- topk threshold refinement ----------------
    # invariant: v* (the KSEL-th largest logit) is in (lo, lo + w]
    lo = small.tile([128, 1], F32)
    nc.vector.memset(lo, LO0)
    neg_edges = small.tile([128, 1], F32)
    acc = small.tile([128, 1], F32)
    sel = small.tile([128, 1], F32)
    ssum = small.tile([128, 1], F32)
    sign_scr = pa.tile([128, N], BF16)  # Sign output is never read
    thr_acc = float(2 * KSEL - N)  # acc = #gt - #lt ; acc>=thr <=> #gt>=KSEL
    for r in range(N_ROUNDS):
        wstep = W0 / (128.0 ** (r + 1))
        nc.vector.scalar_tensor_tensor(
            neg_edges, iota128, -wstep, lo, op0=ALU.mult, op1=ALU.subtract)
        nc.scalar.activation(sign_scr, lrep, AF.Sign, bias=neg_edges,
                             scale=1.0, accum_out=acc)
        nc.vector.tensor_single_scalar(sel, acc, thr_acc, op=ALU.is_ge)
        nc.gpsimd.partition_all_reduce(ssum, sel, channels=128,
                                       reduce_op=bass_isa.ReduceOp.add)
        # lo += ssum * wstep   (bit-identical to the edge it selects)
        nc.vector.scalar_tensor_tensor(
            lo, ssum, wstep, lo, op0=ALU.mult, op1=ALU.add)

    # mask row + gated-mask row, replicated gated mask for Q
    m01 = pa.tile([1, N], F32)
    nc.vector.tensor_scalar(m01, logits_sb, lo[0:1, 0:1], None, op0=ALU.is_gt)
    nc.sync.dma_start(m_dram, m01)
    mg_row = pa.tile([1, N], F32)
    nc.vector.tensor_tensor(mg_row, m01, grow, op=ALU.mult)
    mg_rep = big.tile([128, N], F32)
    for g in range(4):
        ps = psum_b.tile([128, 512], F32, tag="bcast")
        nc.tensor.matmul(ps, ones_row, mg_row[:, g * 512:(g + 1) * 512],
                         start=True, stop=True)
        nc.vector.tensor_copy(mg_rep[:, g * 512:(g + 1) * 512], ps)

    # ---------------- compact positions ----------------
    # token t=c*128+j -> dest row (cumsum-1) if selected else TRASH
    m_c = small.tile([16, 128], F32)
    nc.sync.dma_start(m_c, m_dram.rearrange("(c j) -> c j", j=128))
    cum = small.tile([16, 128], F32)
    nc.vector.tensor_tensor_scan(cum, m_c, m_c, 0.0, op0=ALU.add,
                                 op1=ALU.bypass)
    offs_ps = psum_b.tile([16, 16], F32, tag="offs")
    nc.tensor.matmul(offs_ps[:, 0:1], l16e, cum[:, 127:128],
                     start=True, stop=True)
    offs_sb = small.tile([16, 1], F32)
    nc.vector.tensor_copy(offs_sb, offs_ps[:, 0:1])
    cum_g = small.tile([16, 128], F32)
    nc.vector.tensor_scalar(cum_g, cum, offs_sb, None, op0=ALU.add)
    # pos = (cum_g - (TRASH+1))*m + TRASH  -> cum_g-1 if m==1 else TRASH
    pos_f = small.tile([16, 128], F32)
    nc.vector.tensor_tensor(pos_f, cum_g, m_c, op=ALU.mult)
    nc.vector.scalar_tensor_tensor(pos_f, m_c, -float(TRASH + 1), pos_f,
                                   op0=ALU.mult, op1=ALU.add)
    nc.vector.tensor_scalar(pos_f, pos_f, float(TRASH), None, op0=ALU.add)
    nc.vector.tensor_single_scalar(pos_f, pos_f, float(TRASH), op=ALU.min)
    pos_i = small.tile([16, 128], I32)
    nc.vector.tensor_copy(pos_i, pos_f)
    pos_i16 = small.tile([16, 128], I16)
    nc.vector.tensor_copy(pos_i16, pos_i)
    nc.sync.dma_start(pos_dram.rearrange("(c j) -> c j", j=128), pos_i16)
    idxs_sb = small.tile([128, 128], I16)
    for k in range(8):
        nc.sync.dma_start(idxs_sb[k * 16:(k + 1) * 16, :],
                          pos_dram.rearrange("(a q) -> q a", q=16))
    pr_ctx.close()

    # ---------------- K/V projection (token-major) ----------------
    pkv = pkv_ctx.enter_context(tc.tile_pool(name="pkv", bufs=1))
    psum_kv = pkv_ctx.enter_context(
        tc.tile_pool(name="psum_kv", bufs=3, space="PSUM"))
    kv_sb = pkv.tile([128, 16, KVW], BF16)
    nc.vector.memset(kv_sb[:, :, VOFF:], 0.0)
    for g in range(4):
        nc.vector.memset(kv_sb[:, :, VOFF + g * VSTR + 64:
                               VOFF + g * VSTR + 65], 1.0)
    for tc_ in range(16):
        ps = psum_kv.tile([128, 512], F32, tag="kv_ps", name=f"kv_ps{tc_}")
        ts_ = slice(tc_ * 128, (tc_ + 1) * 128)
        for cc in range(CC):
            nc.tensor.matmul(ps, xhi_sb[:, cc, ts_], wkv_sb[:, cc, :],
                             start=(cc == 0), stop=(cc == CC - 1))
        gsc = g_v[:, tc_:tc_ + 1]
        nc.vector.tensor_scalar(kv_sb[:, tc_, 0:256], ps[:, 0:256], gsc, None,
                                op0=ALU.mult)
        for g in range(4):
            nc.vector.tensor_scalar(
                kv_sb[:, tc_, VOFF + g * VSTR:VOFF + g * VSTR + 64],
                ps[:, 256 + g * 64:256 + (g + 1) * 64], gsc, None,
                op0=ALU.mult)

    # scatter selected rows into compact kv_dram (masked -> trash row)
    nc.gpsimd.dma_scatter_add(
        out_ap=kv_dram[:, :], in_ap=kv_sb[:, :, :], idxs_ap=idxs_sb[:, :],
        num_idxs=N, num_idxs_reg=N, elem_size=KVW)
    pkv_ctx.close()

    # compact K^T (dims-major via DMA transpose) and V (token-major, direct)
    kt_sb = big.tile([128, 2, NCPT], BF16)
    for s in range(2):
        nc.sync.dma_start_transpose(
            kt_sb[:, s, :], kv_dram[0:NCPT, s * 128:(s + 1) * 128])
    v_sb = big.tile([128, KCC, 4 * VSTR], BF16)
    nc.sync.dma_start(
        v_sb, kv_dram[0:NCPT, VOFF:].rearrange("(c p) d -> p c d", p=128))

    # ---------------- Q projection ----------------
    pq_ctx = contextlib.ExitStack()
    psum_q = pq_ctx.enter_context(
        tc.tile_pool(name="psum_q", bufs=3, space="PSUM"))
    qt_sb = big.tile([128, H // 2, NQ], BF16)
    for j in range(H // 2):
        for g in range(NQ // 512):
            ps = psum_q.tile([128, 512], F32, tag="q_ps", name=f"q_ps{j}_{g}")
            qs = slice(g * 512, (g + 1) * 512)
            for cc in range(CC):
                nc.tensor.matmul(ps, wq_sb[:, cc, j * 128:(j + 1) * 128],
                                 xhi_sb[:, cc, qs],
                                 start=(cc == 0), stop=(cc == CC - 1))
            nc.vector.tensor_tensor(qt_sb[:, j, qs], ps, mg_rep[:, qs],
                                    op=ALU.mult)
    pq_ctx.close()
    pa_ctx.close()
    pw_ctx.close()
    px_ctx.close()

    # ---------------- attention ----------------
    ph2_ctx = contextlib.ExitStack()
    ph2 = ph2_ctx.enter_context(tc.tile_pool(name="ph2", bufs=1))
    wo_sb = ph2.tile([128, CC, C], BF16)
    for cc in range(CC):
        nc.sync.dma_start(wo_sb[:, cc, :], wo[cc * 128:(cc + 1) * 128, :])
    oT_bf = big.tile([128, CC, NQ], BF16)
    denom_sb = ph2.tile([16, NQ], F32)

    patt_ctx = contextlib.ExitStack()
    scr_pool = patt_ctx.enter_context(tc.tile_pool(name="scr_pool", bufs=2))
    p_pool = patt_ctx.enter_context(tc.tile_pool(name="p_pool", bufs=3))
    lg_pool = patt_ctx.enter_context(
        tc.tile_pool(name="lg_pool", bufs=2, space="PSUM"))
    att_pool = patt_ctx.enter_context(
        tc.tile_pool(name="att_pool", bufs=1, space="PSUM"))
    oT_f = ph2.tile([128, CC, NQ], F32)

    inv_sqrt_dh = float(1.0 / np.sqrt(DH))
    for j in range(H // 2):
        sl_ = j // 4
        heads = (ORDER[2 * j], ORDER[2 * j + 1])
        att_ps = [att_pool.tile([65, NQ], F32, tag=f"att{m}",
                                name=f"att{j}_{m}") for m in range(2)]
        pend = []  # pipelined attv matmuls: one kc behind the exp
        for kc in range(KCC):
            ks = slice(kc * 128, (kc + 1) * 128)
            for m in range(2):
                h = heads[m]
                pb = slice(m * 64, (m + 1) * 64)
                lg = lg_pool.tile([128, NQ], F32, tag="lg",
                                  name=f"lg{j}_{kc}_{m}")
                for g in range(NQ // 512):
                    gs = slice(g * 512, (g + 1) * 512)
                    nc.tensor.matmul(lg[:, gs], kt_sb[pb, sl_, ks],
                                     qt_sb[pb, j, gs], start=True, stop=True)
                p_t = p_pool.tile([128, NQ], BF16, tag="p_t",
                                  name=f"p_{j}_{kc}_{m}")
                nc.scalar.activation(p_t, lg, AF.Exp, scale=inv_sqrt_dh)
                for f in pend:
                    f()
                pend = []

                def attv(p_t=p_t, kc=kc, m=m, h=h):
                    gv = h // 4
                    vsl = v_sb[:, kc, gv * VSTR:gv * VSTR + 65]
                    for g in range(NQ // 512):
                        gs = slice(g * 512, (g + 1) * 512)
                        nc.tensor.matmul(att_ps[m][:, gs], vsl, p_t[:, gs],
                                         start=(kc == 0), stop=(kc == KCC - 1))

                pend.append(attv)
        for f in pend:
            f()
        # fast evict: psum -> sbuf scratch; denom row out; numerator
        # unscaled into oT (scaled once after all pairs)
        for m in range(2):
            h = heads[m]
            scr65 = scr_pool.tile([65, NQ], F32R, tag="scr65",
                                  name=f"scr65_{j}_{m}")
            nc.vector.tensor_copy(scr65, att_ps[m].bitcast(F32R))
            nc.sync.dma_start(denom_sb[h:h + 1, :],
                              scr65[64:65, :].bitcast(F32))
            if m == 0:
                nc.vector.tensor_copy(oT_f[0:64, j, :].bitcast(F32R),
                                      scr65[0:64, :])
            else:
                nc.sync.dma_start(oT_f[64:128, j, :].bitcast(F32R),
                                  scr65[0:64, :])

    # oT scaling: denom += ZCORR, batched reciprocal, per-slot broadcast of
    # the two relevant denom rows via a tiny sel8 matmul, multiply -> bf16.
    nc.vector.tensor_scalar(denom_sb, denom_sb, ZCORR, None, op0=ALU.add)
    rec16 = ph2.tile([16, NQ], F32R)
    rec16_f = ph2.tile([16, NQ], F32)
    with nc.allow_low_precision(reason="2e-5 rel err << output tolerance"):
        nc.vector.reciprocal_approx_fast(out=rec16_f, in_=denom_sb)
    nc.vector.tensor_copy(rec16, rec16_f)
    for dd in range(CC):
        for g in range(NQ // 512):
            bps = lg_pool.tile([128, 512], F32, tag="lg", name=f"bps{dd}_{g}")
            nc.tensor.matmul(bps, sel8[:, dd, :], rec16[:, g * 512:(g + 1) * 512],
                             start=True, stop=True)
            gs = slice(g * 512, (g + 1) * 512)
            nc.vector.tensor_tensor(oT_bf[:, dd, gs],
                                    oT_f[:, dd, gs], bps.bitcast(F32R),
                                    op=ALU.mult)
    patt_ctx.close()

    # ---------------- output projection ----------------
    ph3_ctx = contextlib.ExitStack()
    psum3 = ph3_ctx.enter_context(
        tc.tile_pool(name="psum3", bufs=4, space="PSUM"))
    out_pool = ph3_ctx.enter_context(tc.tile_pool(name="out_pool", bufs=2))
    for tt in range(NQ // 128):
        out_sb = out_pool.tile([128, C], F32, tag="out_sb", name=f"out_sb{tt}")
        for og in range(C // 512):
            ps = psum3.tile([128, 512], F32, tag="out_ps",
                            name=f"out_ps{tt}_{og}")
            for dd in range(CC):
                nc.tensor.matmul(ps, oT_bf[:, dd, tt * 128:(tt + 1) * 128],
                                 wo_sb[:, dd, og * 512:(og + 1) * 512],
                                 start=(dd == 0), stop=(dd == CC - 1))
            nc.scalar.copy(out_sb[:, og * 512:(og + 1) * 512], ps)
        nc.sync.dma_start(out_d[tt * 128:(tt + 1) * 128, :], out_sb)
    ph3_ctx.close()
    ph2_ctx.close()


_NC = None


def build_program():
    global _NC
    if _NC is not None:
        return _NC
    from contextlib import ExitStack

    nc = bacc.Bacc("TRN2", target_bir_lowering=False, debug=False,
                   num_devices=8)
    io = {
        "xhi": nc.dram_tensor("xhi", (C, N), BF16, kind="ExternalInput").ap(),
        "xlo": nc.dram_tensor("xlo", (C, N), BF16, kind="ExternalInput").ap(),
        "rwh": nc.dram_tensor("rwh", (C, 1), BF16, kind="ExternalInput").ap(),
        "rwl": nc.dram_tensor("rwl", (C, 1), BF16, kind="ExternalInput").ap(),
        "wq": nc.dram_tensor("wq", (C, C), BF16, kind="ExternalInput").ap(),
        "wkv": nc.dram_tensor("wkv", (C, 512), BF16,
                              kind="ExternalInput").ap(),
        "wo": nc.dram_tensor("wo", (C, C), BF16, kind="ExternalInput").ap(),
        "sel8": nc.dram_tensor("sel8", (16, CC, 128), F32,
                               kind="ExternalInput").ap(),
        "l16e": nc.dram_tensor("l16e", (16, 16), F32,
                               kind="ExternalInput").ap(),
        "out": nc.dram_tensor("out", (NQ, C), F32, kind="ExternalOutput").ap(),
    }
    with TileContext(nc) as tc:
        with ExitStack() as ctx:
            _emit(nc, tc, ctx, io)
    nc.compile()
    _NC = nc
    return nc


def _permute_cols(w):
    """Column-permute so slot j's 128 cols = heads (ORDER[2j], ORDER[2j+1])."""
    w = np.asarray(w, np.float32).reshape(C, H, DH)
    return np.ascontiguousarray(w[:, ORDER, :].reshape(C, H * DH))


def make_in_maps(x, router_w, wq, wk, wv, wo):
    import ml_dtypes

    BF = ml_dtypes.bfloat16
    wq_p = _permute_cols(wq).astype(BF)
    # wo rows follow the same head order as oT slots
    wo_p = np.ascontiguousarray(
        np.asarray(wo, np.float32).reshape(H, DH, C)[ORDER].reshape(C, C)
    ).astype(BF)
    wkv = np.concatenate(
        [np.asarray(wk, np.float32), np.asarray(wv, np.float32)],
        axis=1).astype(BF)
    rw = np.asarray(router_w, np.float32)
    rwh = rw.astype(BF)
    rwl = (rw - rwh.astype(np.float32)).astype(BF)
    sel8 = np.zeros((16, CC, 128), np.float32)
    for dd in range(CC):
        for p in range(128):
            sel8[ORDER[2 * dd + p // 64], dd, p] = 1.0
    l16e = np.triu(np.ones((16, 16), np.float32), 1)  # l16e[k,m]=1 iff k<m
    in_maps = []
    for core in range(8):
        b, h = core // 2, core % 2
        xT = np.ascontiguousarray(
            np.roll(np.asarray(x[b], np.float32).T, -h * NQ, axis=1))
        xT_hi = xT.astype(BF)
        xT_lo = (xT - xT_hi.astype(np.float32)).astype(BF)
        in_maps.append({
            "xhi": xT_hi,
            "xlo": xT_lo,
            "rwh": rwh,
            "rwl": rwl,
            "wq": wq_p,
            "wkv": wkv,
            "wo": wo_p,
            "sel8": sel8,
            "l16e": l16e,
        })
    return in_maps


def _numpy_fallback(x, router_w, router_b, wq, bq, wk, bk, wv, bv, wo, bo):
    x = np.asarray(x, np.float32)
    gate = 1.0 / (1.0 + np.exp(-(x @ router_w + router_b)))
    xg = x * gate
    scores = gate[..., 0]
    idx = np.argsort(-scores, axis=-1, kind="stable")[:, :KSEL]
    mask = np.zeros((x.shape[0], x.shape[1]), np.float32)
    np.put_along_axis(mask, idx, 1.0, axis=1)
    xg = xg * mask[..., None]
    q = (xg @ wq + bq).reshape(B, N, H, DH)
    kk = np.repeat((xg @ wk + bk).reshape(B, N, HKV, DH), H // HKV, axis=2)
    v = np.repeat((xg @ wv + bv).reshape(B, N, HKV, DH), H // HKV, axis=2)
    att = np.einsum("bqhd,bkhd->bhqk", q, kk) / np.float32(np.sqrt(DH))
    att = att - att.max(-1, keepdims=True)
    att = np.exp(att)
    att = att / att.sum(-1, keepdims=True)
    o = np.einsum("bhqk,bkhd->bqhd", att, v).reshape(B, N, C)
    return (o @ wo + bo).astype(np.float32)


def kernel(x, router_w, router_b, wq, bq, wk, bk, wv, bv, wo, bo):
    x = np.asarray(x)
    biases = [router_b, bq, bk, bv, bo]
    if any(float(np.abs(np.asarray(t)).max()) != 0.0 for t in biases):
        # The device program folds away the (identically zero) biases; fall
        # back to an exact host implementation if that assumption breaks.
        return _numpy_fallback(x, router_w, router_b, wq, bq, wk, bk, wv, bv,
                               wo, bo)

    from concourse import bass_utils

    nc = build_program()
    in_maps = make_in_maps(x, router_w, wq, wk, wv, wo)
    res = bass_utils.run_bass_kernel_spmd(nc, in_maps, core_ids=list(range(8)))
    out = np.empty((B, N, C), np.float32)
    for core in range(8):
        b, h = core // 2, core % 2
        out[b, h * NQ:(h + 1) * NQ, :] = res.results[core]["out"]
    return out


# revision 27
# speedup vs baseline: 1.4174x; 1.4174x over previous
"""Trainium2 Bass kernel for MIGAttention (topk token masking + GQA attention).

Shapes (hardcoded): B=4, N=2048, C=1024, H=16 heads, HKV=4 kv-heads, DH=64,
keep-ratio 0.7 -> k = 1433 selected tokens per batch row.

Sharding: 8 cores = (batch b in 0..3) x (query-half h in 0..1).  Each core
receives x[b].T (bf16 hi/lo split) with token columns rolled by h*1024 so its
own query half occupies columns 0..1023 -> a single SPMD program for all
cores.

Per core: split-bf16 router (fp32-accurate logits), 4-round threshold
refinement reproducing the exact top-k mask, then an ON-CHIP compaction of
the 1433 selected tokens' K/V rows into 1536 dense rows: per destination
chunk, small 0/1 selection-matrix blocks (pos == dst, built on DVE) multiply
the token-major KV staging tile on the PE (a destination row can only come
from source tokens at most 615 positions ahead, so 6 source chunks suffice).
GQA attention (1024 queries x 1536 compact keys, exp on ScalarE) follows,
with masked keys' exp(0)=1 denominator contribution added analytically
(+615).  Q projections for later head-slots are interleaved into the
attention loop to keep the PE HAM clock warm.
"""

import contextlib
import sys

import numpy as np

if "/opt/trn_rl_repo" not in sys.path:
    sys.path.insert(0, "/opt/trn_rl_repo")

import concourse.bass as bass  # noqa: F401
import concourse.bass_isa as bass_isa
import concourse.mybir as mybir
from concourse import bacc
from concourse.tile import TileContext

F32 = mybir.dt.float32
F32R = mybir.dt.float32r
BF16 = mybir.dt.bfloat16
I32 = mybir.dt.int32
AF = mybir.ActivationFunctionType
ALU = mybir.AluOpType

B, N, C = 4, 2048, 1024
H, HKV, DH = 16, 4, 64
NQ = N // 2           # queries per core
KSEL = 1433           # max(1, int(N * 0.7))
CC = C // 128         # contraction chunks (8)
NCPT = 1536           # compact (padded) key count
KCC = NCPT // 128     # compact key chunks (12)
TRASH = NCPT          # position assigned to masked tokens (matches no dst)
KVW = 640             # kv row: K(4g x 64) | 4 x (64 v, 1 one, 31 zero)
VOFF, VSTR = 256, 96
SRCW = 6              # src chunks per dst chunk (ceil((615+127)/128)+1)
N_ROUNDS = 4          # topk threshold refinement rounds
LO0, W0 = -4.0, 8.0   # initial logit search interval (logit std ~0.65)
ZCORR = float(N - KSEL)  # masked keys' exp(0) denominator contribution
# slot j holds q-heads (ORDER[2j], ORDER[2j+1]) on partition halves; the two
# heads come from kv groups (2s, 2s+1), s=j//4, matching the two 64-partition
# halves of kt slot s.
ORDER = [0, 4, 1, 5, 2, 6, 3, 7, 8, 12, 9, 13, 10, 14, 11, 15]


def _emit(nc, tc, ctx, io):
    xhi, xlo, rwh, rwl, wq, wkv, wo, sel8_d, out_d = (
        io["xhi"], io["xlo"], io["rwh"], io["rwl"], io["wq"], io["wkv"],
        io["wo"], io["sel8"], io["out"])

    # ---------------- long-lived pools ----------------
    const = ctx.enter_context(tc.tile_pool(name="const", bufs=1))
    small = ctx.enter_context(tc.tile_pool(name="small", bufs=1))
    big = ctx.enter_context(tc.tile_pool(name="big", bufs=1))
    dram = ctx.enter_context(tc.tile_pool(name="dram", bufs=1, space="DRAM"))

    # stack-ordered phase pools (created in close-reverse order)
    px_ctx = contextlib.ExitStack()    # xhi (through attention: Q proj)
    pw_ctx = contextlib.ExitStack()    # wq, wkv
    pa_ctx = contextlib.ExitStack()    # router/refinement/kv staging scratch
    pkv_ctx = contextlib.ExitStack()   # KV-projection psum
    pr_ctx = contextlib.ExitStack()    # router+bcast psum
    plo_ctx = contextlib.ExitStack()   # xlo (router only)
    pc_ctx = contextlib.ExitStack()    # compaction psum

    # ---------------- constants (all on-chip) ----------------
    ones_row = const.tile([1, 128], F32)
    nc.vector.memset(ones_row, 1.0)
    iota128_i = const.tile([128, 1], I32)
    nc.gpsimd.iota(iota128_i, pattern=[[0, 1]], base=1, channel_multiplier=1)
    iota128 = const.tile([128, 1], F32)   # per-partition 1..128
    nc.vector.tensor_copy(iota128, iota128_i)
    iota0 = const.tile([128, 1], F32)     # per-partition 0..127
    nc.vector.tensor_scalar(iota0, iota128, -1.0, None, op0=ALU.add)
    iotaF_i = const.tile([128, 128], I32)
    nc.gpsimd.iota(iotaF_i, pattern=[[1, 128]], base=0, channel_multiplier=0)
    iotaF = const.tile([128, 128], F32)   # free-axis 0..127, all partitions
    nc.vector.tensor_copy(iotaF, iotaF_i)
    # L128[k, m] = 1 iff k <= m  (f32: pairs with the f32 mask in the MM)
    l128 = const.tile([128, 128], F32)
    nc.vector.tensor_scalar(l128, iotaF, iota0, None, op0=ALU.is_ge)
    # identities for PE transposes (dtype must match the transposed data)
    ident = const.tile([128, 128], BF16)
    nc.vector.tensor_scalar(ident, iotaF, iota0, None, op0=ALU.is_equal)
    identf = const.tile([128, 128], F32)
    nc.vector.tensor_scalar(identf, iotaF, iota0, None, op0=ALU.is_equal)
    sel8 = const.tile([16, CC, 128], F32R)
    nc.sync.dma_start(sel8, sel8_d.bitcast(F32R))

    # ---------------- DRAM scratch ----------------
    m_dram = dram.tile([N], F32)
    g_dram = dram.tile([N], F32)

    # ---------------- big loads ----------------
    px = px_ctx.enter_context(tc.tile_pool(name="px", bufs=1))
    pw = pw_ctx.enter_context(tc.tile_pool(name="pw", bufs=1))
    pa = pa_ctx.enter_context(tc.tile_pool(name="pa", bufs=1))
    psum_r = pr_ctx.enter_context(tc.tile_pool(name="psum_r", bufs=1,
                                               space="PSUM"))
    psum_b = pr_ctx.enter_context(tc.tile_pool(name="psum_b", bufs=2,
                                               space="PSUM"))
    psum_kv = pkv_ctx.enter_context(
        tc.tile_pool(name="psum_kv", bufs=2, space="PSUM"))
    plo = plo_ctx.enter_context(tc.tile_pool(name="plo", bufs=1))

    xhi_sb = px.tile([128, CC, N], BF16)
    xlo_sb = plo.tile([128, 4, N], BF16)
    for cc in range(CC):
        sl = slice(cc * 128, (cc + 1) * 128)
        nc.sync.dma_start(xhi_sb[:, cc, :], xhi[sl, :])
        if cc < 4:
            nc.sync.dma_start(xlo_sb[:, cc, :], xlo[sl, :])
    rwh_sb = small.tile([128, CC], BF16)
    rwl_sb = small.tile([128, CC], BF16)
    for cc in range(CC):
        sl = slice(cc * 128, (cc + 1) * 128)
        nc.sync.dma_start(rwh_sb[:, cc:cc + 1], rwh[sl, :])
        nc.sync.dma_start(rwl_sb[:, cc:cc + 1], rwl[sl, :])
    wkv_sb = pw.tile([128, CC, 512], BF16)
    for cc in range(CC):
        nc.sync.dma_start(wkv_sb[:, cc, :], wkv[cc * 128:(cc + 1) * 128, :])

    # ---------------- router: logits in split-bf16 ----------------
    rps = psum_r.tile([1, N], F32)
    for cc in range(CC):
        for g in range(4):
            gs = slice(g * 512, (g + 1) * 512)
            nc.tensor.matmul(rps[:, gs], rwh_sb[:, cc:cc + 1],
                             xhi_sb[:, cc, gs], start=(cc == 0), stop=False)
            nc.tensor.matmul(rps[:, gs], rwl_sb[:, cc:cc + 1],
                             xhi_sb[:, cc, gs], start=False, stop=False)
            nc.tensor.matmul(rps[:, gs], rwh_sb[:, cc:cc + 1],
                             xlo_sb[:, cc % 4, gs], start=False,
                             stop=(cc == CC - 1))
        if cc + 4 < CC:
            nc.sync.dma_start(xlo_sb[:, cc % 4, :],
                              xlo[(cc + 4) * 128:(cc + 5) * 128, :])
    logits_sb = pa.tile([1, N], F32)
    nc.vector.tensor_copy(logits_sb, rps)
    plo_ctx.close()

    # replicate logits across partitions (fp32 K=1 broadcast matmuls)
    lrep = pa.tile([128, N], F32)
    for g in range(4):
        ps = psum_b.tile([128, 512], F32, tag="bcast")
        nc.tensor.matmul(ps, ones_row, logits_sb[:, g * 512:(g + 1) * 512],
                         start=True, stop=True)
        nc.vector.tensor_copy(lrep[:, g * 512:(g + 1) * 512], ps)

    # ---------------- topk threshold refinement ----------------
    # invariant: v* (the KSEL-th largest logit) is in (lo, lo + w]
    lo = small.tile([128, 1], F32)
    nc.vector.memset(lo, LO0)
    neg_edges = small.tile([128, 1], F32)
    acc = small.tile([128, 1], F32)
    sel = small.tile([128, 1], F32)
    ssum = small.tile([128, 1], F32)
    sign_scr = pa.tile([128, N], BF16)  # Sign output is never read
    thr_acc = float(2 * KSEL - N)  # acc = #gt - #lt ; acc>=thr <=> #gt>=KSEL
    for r in range(N_ROUNDS):
        wstep = W0 / (128.0 ** (r + 1))
        nc.vector.scalar_tensor_tensor(
            neg_edges, iota128, -wstep, lo, op0=ALU.mult, op1=ALU.subtract)
        nc.scalar.activation(sign_scr, lrep, AF.Sign, bias=neg_edges,
                             scale=1.0, accum_out=acc)
        nc.vector.tensor_single_scalar(sel, acc, thr_acc, op=ALU.is_ge)
        nc.gpsimd.partition_all_reduce(ssum, sel, channels=128,
                                       reduce_op=bass_isa.ReduceOp.add)
        # lo += ssum * wstep   (bit-identical to the edge it selects)
        nc.vector.scalar_tensor_tensor(
            lo, ssum, wstep, lo, op0=ALU.mult, op1=ALU.add)

    # gate row + token-major gate (via padded DVE transpose)
    grow = pa.tile([1, N], F32)
    nc.scalar.activation(grow, logits_sb, AF.Sigmoid)
    nc.sync.dma_start(g_dram, grow)
    g_c = pa.tile([16, 128], F32)
    nc.sync.dma_start(g_c, g_dram.rearrange("(c j) -> c j", j=128))
    g_ps = psum_b.tile([128, 16], F32, tag="bcast")
    nc.tensor.transpose(g_ps, g_c, identf[0:16, 0:16])
    g_v = pa.tile([128, 16], F32)         # gate, token-major
    nc.vector.tensor_copy(g_v, g_ps)

    # ---------------- K/V projection (token-major, gate-scaled) ---------
    # (emitted after the refinement so its eviction DVE ops don't queue in
    # front of the refinement's; the matmuls themselves have no dependency
    # on it and overlap the refinement on the PE)
    kv_sb = pa.tile([128, 16, KVW], BF16)
    nc.vector.memset(kv_sb[:, :, VOFF:], 0.0)
    for g in range(4):
        nc.vector.memset(kv_sb[:, :, VOFF + g * VSTR + 64:
                               VOFF + g * VSTR + 65], 1.0)
    for tc_ in range(16):
        ps = psum_kv.tile([128, 512], F32, tag="kv_ps", name=f"kv_ps{tc_}")
        ts_ = slice(tc_ * 128, (tc_ + 1) * 128)
        for cc in range(CC):
            nc.tensor.matmul(ps, xhi_sb[:, cc, ts_], wkv_sb[:, cc, :],
                             start=(cc == 0), stop=(cc == CC - 1))
        gsc = g_v[:, tc_:tc_ + 1]
        nc.vector.tensor_scalar(kv_sb[:, tc_, 0:256], ps[:, 0:256], gsc, None,
                                op0=ALU.mult)
        for g in range(4):
            nc.vector.tensor_scalar(
                kv_sb[:, tc_, VOFF + g * VSTR:VOFF + g * VSTR + 64],
                ps[:, 256 + g * 64:256 + (g + 1) * 64], gsc, None,
                op0=ALU.mult)
    pkv_ctx.close()

    # mask row; replicated (mask*gate) for Q-side masking
    m01 = pa.tile([1, N], F32)
    nc.vector.tensor_scalar(m01, logits_sb, lo[0:1, 0:1], None, op0=ALU.is_gt)
    nc.sync.dma_start(m_dram, m01)
    mg_row = pa.tile([1, N], F32)
    nc.vector.tensor_tensor(mg_row, m01, grow, op=ALU.mult)
    mg_rep = big.tile([128, N], F32)
    for g in range(4):
        ps = psum_b.tile([128, 512], F32, tag="bcast")
        nc.tensor.matmul(ps, ones_row, mg_row[:, g * 512:(g + 1) * 512],
                         start=True, stop=True)
        nc.vector.tensor_copy(mg_rep[:, g * 512:(g + 1) * 512], ps)

    # ---------------- compact destination positions (all on-chip) -------
    # pos[t] = rank of t among selected (0-based) if selected else TRASH
    m_c = pa.tile([16, 128], F32)
    nc.sync.dma_start(m_c, m_dram.rearrange("(c j) -> c j", j=128))
    m_ps = psum_b.tile([128, 16], F32, tag="bcast")
    nc.tensor.transpose(m_ps, m_c, identf[0:16, 0:16])
    m_v = pa.tile([128, 16], F32)         # mask, token-major
    nc.vector.tensor_copy(m_v, m_ps)
    # within-chunk inclusive cumsum along partitions via L128 matmul
    cum_ps = psum_b.tile([128, 16], F32, tag="bcast")
    nc.tensor.matmul(cum_ps, l128, m_v, start=True, stop=True)
    cum_sb = pa.tile([128, 16], F32)
    nc.vector.tensor_copy(cum_sb, cum_ps)
    # exclusive chunk offsets: scan totals row, subtract, broadcast
    tot_row = pa.tile([1, 16], F32)
    nc.sync.dma_start(tot_row, cum_sb[127:128, :])
    incl_row = pa.tile([1, 16], F32)
    nc.vector.tensor_tensor_scan(incl_row, tot_row, tot_row, 0.0,
                                 op0=ALU.add, op1=ALU.bypass)
    excl_row = pa.tile([1, 16], F32)
    nc.vector.tensor_tensor(excl_row, incl_row, tot_row, op=ALU.subtract)
    offs_ps = psum_b.tile([128, 16], F32, tag="bcast")
    nc.tensor.matmul(offs_ps, ones_row, excl_row, start=True, stop=True)
    pos_v = pa.tile([128, 16], F32)
    nc.vector.tensor_tensor(pos_v, cum_sb, offs_ps, op=ALU.add)
    # pos = (cum-1) if selected else TRASH; clamp (safety)
    nc.vector.tensor_tensor(pos_v, pos_v, m_v, op=ALU.mult)
    nc.vector.scalar_tensor_tensor(pos_v, m_v, -float(TRASH + 1), pos_v,
                                   op0=ALU.mult, op1=ALU.add)
    nc.vector.tensor_scalar(pos_v, pos_v, float(TRASH), None, op0=ALU.add)
    nc.vector.tensor_single_scalar(pos_v, pos_v, float(TRASH), op=ALU.min)
    pr_ctx.close()

    # ---------------- KV compaction via selection-matrix matmuls --------
    # dst row a comes from src token t with pos[t]==a; t-a in [0, 615], so
    # dst chunk dc draws only from src chunks dc..dc+SRCW-1.
    kv_cmp = big.tile([128, KCC, KVW], BF16)
    psum_c = pc_ctx.enter_context(
        tc.tile_pool(name="psum_c", bufs=2, space="PSUM"))
    psum_t = pc_ctx.enter_context(
        tc.tile_pool(name="psum_t", bufs=2, space="PSUM"))
    pblk_pool = pc_ctx.enter_context(tc.tile_pool(name="pblk", bufs=3))
    for dc in range(KCC):
        tcs = [t for t in range(dc, min(dc + SRCW, 16))]
        cps = psum_c.tile([128, KVW], F32, tag="cmp", name=f"cmp{dc}")
        for i, tc_ in enumerate(tcs):
            pshift = pblk_pool.tile([128, 1], F32, tag="pshift",
                                    name=f"pshift{dc}_{tc_}")
            nc.vector.tensor_scalar(pshift, pos_v[:, tc_:tc_ + 1],
                                    -float(dc * 128), None, op0=ALU.add)
            pblk = pblk_pool.tile([128, 128], BF16, tag="pblk",
                                  name=f"pblk{dc}_{tc_}")
            nc.vector.tensor_scalar(pblk, iotaF, pshift, None,
                                    op0=ALU.is_equal)
            nc.tensor.matmul(cps[:, 0:512], pblk, kv_sb[:, tc_, 0:512],
                             start=(i == 0), stop=(i == len(tcs) - 1))
            nc.tensor.matmul(cps[:, 512:KVW], pblk, kv_sb[:, tc_, 512:KVW],
                             start=(i == 0), stop=(i == len(tcs) - 1))
        nc.vector.tensor_copy(kv_cmp[:, dc, :], cps)

    # compact K^T (dims-major) via PE transposes
    kt_sb = big.tile([128, 2, NCPT], BF16)
    for s in range(2):
        for dc in range(KCC):
            tps = psum_t.tile([128, 128], BF16, tag="tp",
                              name=f"tp{s}_{dc}")
            nc.tensor.transpose(tps, kv_cmp[:, dc, s * 128:(s + 1) * 128],
                                ident)
            nc.vector.tensor_copy(kt_sb[:, s, dc * 128:(dc + 1) * 128], tps)

    # Q projection, slot 0 (rest interleaved into the attention loop)
    qt_sb = big.tile([128, H // 2, NQ], BF16)

    def qproj(j, g, pool):
        ps = pool.tile([128, 512], F32, tag="q_ps", name=f"q_ps{j}_{g}")
        qs = slice(g * 512, (g + 1) * 512)
        for cc in range(CC):
            nc.tensor.matmul(ps, wq_sb[:, cc, j * 128:(j + 1) * 128],
                             xhi_sb[:, cc, qs],
                             start=(cc == 0), stop=(cc == CC - 1))
        nc.vector.tensor_tensor(qt_sb[:, j, qs], ps, mg_rep[:, qs],
                                op=ALU.mult)

    pc_ctx.close()
    pa_ctx.close()

    # ---------------- attention ----------------
    ph2_ctx = contextlib.ExitStack()
    ph2 = ph2_ctx.enter_context(tc.tile_pool(name="ph2", bufs=1))
    wq_sb = ph2.tile([128, CC, C], BF16)
    for cc in range(CC):
        nc.sync.dma_start(wq_sb[:, cc, :], wq[cc * 128:(cc + 1) * 128, :])
    wo_sb = ph2.tile([128, CC, C], BF16)
    for cc in range(CC):
        nc.sync.dma_start(wo_sb[:, cc, :], wo[cc * 128:(cc + 1) * 128, :])
    oT_bf = big.tile([128, CC, NQ], BF16)
    denom_sb = ph2.tile([16, NQ], F32)
    den_stage = ph2.tile([65, NQ], F32)   # row 64 reused per head
    shift_stage = ph2.tile([64, NQ], BF16)

    patt_ctx = contextlib.ExitStack()
    p_pool = patt_ctx.enter_context(tc.tile_pool(name="p_pool", bufs=3))
    lg_pool = patt_ctx.enter_context(
        tc.tile_pool(name="lg_pool", bufs=2, space="PSUM"))
    att_pool = patt_ctx.enter_context(
        tc.tile_pool(name="att_pool", bufs=1, space="PSUM"))
    qp_pool = patt_ctx.enter_context(
        tc.tile_pool(name="qp_pool", bufs=2, space="PSUM"))

    inv_sqrt_dh = float(1.0 / np.sqrt(DH))
    for g in range(NQ // 512):
        qproj(0, g, qp_pool)
    for j in range(H // 2):
        sl_ = j // 4
        for m in range(2):
            h = ORDER[2 * j + m]
            gv = h // 4
            pb = slice(m * 64, (m + 1) * 64)
            att_ps = att_pool.tile([65, NQ], F32, tag="att",
                                   name=f"att{j}_{m}")
            pend = []  # pipelined attv matmuls: one kc behind the exp
            for kc in range(KCC):
                ks = slice(kc * 128, (kc + 1) * 128)
                lg = lg_pool.tile([128, NQ], F32, tag="lg",
                                  name=f"lg{j}_{m}_{kc}")
                for g in range(NQ // 512):
                    gs = slice(g * 512, (g + 1) * 512)
                    nc.tensor.matmul(lg[:, gs], kt_sb[pb, sl_, ks],
                                     qt_sb[pb, j, gs], start=True, stop=True)
                p_t = p_pool.tile([128, NQ], BF16, tag="p_t",
                                  name=f"p_{j}_{m}_{kc}")
                nc.scalar.activation(p_t, lg, AF.Exp, scale=inv_sqrt_dh)
                for f in pend:
                    f()
                pend = []

                def attv(p_t=p_t, kc=kc, att_ps=att_ps, gv=gv):
                    vsl = kv_cmp[:, kc, VOFF + gv * VSTR:VOFF + gv * VSTR + 65]
                    for g in range(NQ // 512):
                        gs = slice(g * 512, (g + 1) * 512)
                        nc.tensor.matmul(att_ps[:, gs], vsl, p_t[:, gs],
                                         start=(kc == 0),
                                         stop=(kc == KCC - 1))

                pend.append(attv)
            for f in pend:
                f()
            # interleave the next slot's Q projection (keeps PE busy)
            if j + 1 < H // 2:
                qproj(j + 1, m, qp_pool)
            # evict: numerator (bf16) into oT, denominator row (f32) out
            nc.vector.tensor_copy(den_stage[64:65, :], att_ps[64:65, :])
            nc.sync.dma_start(denom_sb[h:h + 1, :], den_stage[64:65, :])
            if m == 0:
                nc.vector.tensor_copy(oT_bf[0:64, j, :], att_ps[0:64, :])
            else:
                nc.vector.tensor_copy(shift_stage, att_ps[0:64, :])
                nc.sync.dma_start(oT_bf[64:128, j, :], shift_stage)

    # oT scaling: denom += ZCORR, batched reciprocal, per-slot broadcast of
    # the two relevant denom rows via a tiny sel8 matmul, multiply in place.
    nc.vector.tensor_scalar(denom_sb, denom_sb, ZCORR, None, op0=ALU.add)
    rec16_f = ph2.tile([16, NQ], F32)
    with nc.allow_low_precision(reason="2e-5 rel err << output tolerance"):
        nc.vector.reciprocal_approx_fast(out=rec16_f, in_=denom_sb)
    rec16 = ph2.tile([16, NQ], F32R)
    nc.vector.tensor_copy(rec16, rec16_f)
    for dd in range(CC):
        for g in range(NQ // 512):
            bps = lg_pool.tile([128, 512], F32, tag="lg", name=f"bps{dd}_{g}")
            nc.tensor.matmul(bps, sel8[:, dd, :],
                             rec16[:, g * 512:(g + 1) * 512],
                             start=True, stop=True)
            gs = slice(g * 512, (g + 1) * 512)
            nc.vector.tensor_tensor(oT_bf[:, dd, gs],
                                    oT_bf[:, dd, gs], bps,
                                    op=ALU.mult)
    patt_ctx.close()

    # ---------------- output projection ----------------
    ph3_ctx = contextlib.ExitStack()
    psum3 = ph3_ctx.enter_context(
        tc.tile_pool(name="psum3", bufs=4, space="PSUM"))
    out_pool = ph3_ctx.enter_context(tc.tile_pool(name="out_pool", bufs=2))
    for tt in range(NQ // 128):
        out_sb = out_pool.tile([128, C], F32, tag="out_sb", name=f"out_sb{tt}")
        for og in range(C // 512):
            ps = psum3.tile([128, 512], F32, tag="out_ps",
                            name=f"out_ps{tt}_{og}")
            for dd in range(CC):
                nc.tensor.matmul(ps, oT_bf[:, dd, tt * 128:(tt + 1) * 128],
                                 wo_sb[:, dd, og * 512:(og + 1) * 512],
                                 start=(dd == 0), stop=(dd == CC - 1))
            nc.scalar.copy(out_sb[:, og * 512:(og + 1) * 512], ps)
        nc.sync.dma_start(out_d[tt * 128:(tt + 1) * 128, :], out_sb)
    ph3_ctx.close()
    ph2_ctx.close()
    pw_ctx.close()
    px_ctx.close()


_NC = None


def build_program():
    global _NC
    if _NC is not None:
        return _NC
    from contextlib import ExitStack

    nc = bacc.Bacc("TRN2", target_bir_lowering=False, debug=False,
                   num_devices=8)
    io = {
        "xhi": nc.dram_tensor("xhi", (C, N), BF16, kind="ExternalInput").ap(),
        "xlo": nc.dram_tensor("xlo", (C, N), BF16, kind="ExternalInput").ap(),
        "rwh": nc.dram_tensor("rwh", (C, 1), BF16, kind="ExternalInput").ap(),
        "rwl": nc.dram_tensor("rwl", (C, 1), BF16, kind="ExternalInput").ap(),
        "wq": nc.dram_tensor("wq", (C, C), BF16, kind="ExternalInput").ap(),
        "wkv": nc.dram_tensor("wkv", (C, 512), BF16,
                              kind="ExternalInput").ap(),
        "wo": nc.dram_tensor("wo", (C, C), BF16, kind="ExternalInput").ap(),
        "sel8": nc.dram_tensor("sel8", (16, CC, 128), F32,
                               kind="ExternalInput").ap(),
        "out": nc.dram_tensor("out", (NQ, C), F32, kind="ExternalOutput").ap(),
    }
    with TileContext(nc) as tc:
        with ExitStack() as ctx:
            _emit(nc, tc, ctx, io)
    nc.compile()
    _NC = nc
    return nc


def _permute_cols(w):
    """Column-permute so slot j's 128 cols = heads (ORDER[2j], ORDER[2j+1])."""
    w = np.asarray(w, np.float32).reshape(C, H, DH)
    return np.ascontiguousarray(w[:, ORDER, :].reshape(C, H * DH))


def make_in_maps(x, router_w, wq, wk, wv, wo):
    import ml_dtypes

    BF = ml_dtypes.bfloat16
    wq_p = _permute_cols(wq).astype(BF)
    # wo rows follow the same head order as oT slots
    wo_p = np.ascontiguousarray(
        np.asarray(wo, np.float32).reshape(H, DH, C)[ORDER].reshape(C, C)
    ).astype(BF)
    wkv = np.concatenate(
        [np.asarray(wk, np.float32), np.asarray(wv, np.float32)],
        axis=1).astype(BF)
    rw = np.asarray(router_w, np.float32)
    rwh = rw.astype(BF)
    rwl = (rw - rwh.astype(np.float32)).astype(BF)
    sel8 = np.zeros((16, CC, 128), np.float32)
    for dd in range(CC):
        for p in range(128):
            sel8[ORDER[2 * dd + p // 64], dd, p] = 1.0
    in_maps = []
    for core in range(8):
        b, h = core // 2, core % 2
        xT = np.ascontiguousarray(
            np.roll(np.asarray(x[b], np.float32).T, -h * NQ, axis=1))
        xT_hi = xT.astype(BF)
        xT_lo = (xT - xT_hi.astype(np.float32)).astype(BF)
        in_maps.append({
            "xhi": xT_hi,
            "xlo": xT_lo,
            "rwh": rwh,
            "rwl": rwl,
            "wq": wq_p,
            "wkv": wkv,
            "wo": wo_p,
            "sel8": sel8,
        })
    return in_maps


def _numpy_fallback(x, router_w, router_b, wq, bq, wk, bk, wv, bv, wo, bo):
    x = np.asarray(x, np.float32)
    gate = 1.0 / (1.0 + np.exp(-(x @ router_w + router_b)))
    xg = x * gate
    scores = gate[..., 0]
    idx = np.argsort(-scores, axis=-1, kind="stable")[:, :KSEL]
    mask = np.zeros((x.shape[0], x.shape[1]), np.float32)
    np.put_along_axis(mask, idx, 1.0, axis=1)
    xg = xg * mask[..., None]
    q = (xg @ wq + bq).reshape(B, N, H, DH)
    kk = np.repeat((xg @ wk + bk).reshape(B, N, HKV, DH), H // HKV, axis=2)
    v = np.repeat((xg @ wv + bv).reshape(B, N, HKV, DH), H // HKV, axis=2)
    att = np.einsum("bqhd,bkhd->bhqk", q, kk) / np.float32(np.sqrt(DH))
    att = att - att.max(-1, keepdims=True)
    att = np.exp(att)
    att = att / att.sum(-1, keepdims=True)
    o = np.einsum("bhqk,bkhd->bqhd", att, v).reshape(B, N, C)
    return (o @ wo + bo).astype(np.float32)


def kernel(x, router_w, router_b, wq, bq, wk, bk, wv, bv, wo, bo):
    x = np.asarray(x)
    biases = [router_b, bq, bk, bv, bo]
    if any(float(np.abs(np.asarray(t)).max()) != 0.0 for t in biases):
        # The device program folds away the (identically zero) biases; fall
        # back to an exact host implementation if that assumption breaks.
        return _numpy_fallback(x, router_w, router_b, wq, bq, wk, bk, wv, bv,
                               wo, bo)

    from concourse import bass_utils

    nc = build_program()
    in_maps = make_in_maps(x, router_w, wq, wk, wv, wo)
    res = bass_utils.run_bass_kernel_spmd(nc, in_maps, core_ids=list(range(8)))
    out = np.empty((B, N, C), np.float32)
    for core in range(8):
        b, h = core // 2, core % 2
        out[b, h * NQ:(h + 1) * NQ, :] = res.results[core]["out"]
    return out
